# revision 40
# baseline (speedup 1.0000x reference)
"""Criss-Cross Attention (CCA) Trainium2 Bass kernel — v2.

Problem: n=8 images of (c=512, h=128, w=128); per-pixel projections
q,k (64ch) and v (512ch); row + column attention with joint softmax over
the 256 (w + h) logits per pixel (self pixel masked out of the column
branch); out = gamma * att + x.

Sharding: data-parallel over batch — one image per NeuronCore (8 cores).

v2 design notes (all transposes eliminated):
  P1: stream x in 4-row blocks; project q,k channel-major into SBUF;
      project v PIXEL-major (x16 row-slice as matmul stationary, WvT as
      moving) giving [x, c512] tiles, stored to DRAM as v_scr[y][x][c].
  P2: sum pass (query-on-partition e-matmuls, exp, reduce) -> Z;
      nb = -(lnZ - ln gamma) folded into fp16 hi/lo aug rows of q_sb.
      a-pass with SWAPPED operands (k stationary, q moving) so the
      attention maps come out KEY-on-partition and are written straight
      to SBUF — no XBAR transpose DMAs.
  P3: per oc-pair: col branch (v column tiles read from v_scr as 512B
      lines, v-stationary matmuls) -> contiguous fp16 acc [c,x,y];
      row branch matmuls accumulate into PSUM, the col acc is folded in
      by an identity-matmul whose strided MOVING operand does the
      (x<->y) relabel for free on the PE; one fused DVE op adds the
      residual x and gamma*bv, then a straight DMA writes out.
"""

import sys

for _p in ("/opt/trn_rl_repo",):
    if _p not in sys.path:
        sys.path.insert(0, _p)

from contextlib import ExitStack

import numpy as np

from concourse import bacc
import concourse.bass as bass
import concourse.mybir as mybir
import concourse.tile as tile
from concourse.bass_utils import run_bass_kernel_spmd

F32 = mybir.dt.float32
F16 = mybir.dt.float16
AX = mybir.AxisListType
ALU = mybir.AluOpType
AF = mybir.ActivationFunctionType

N_CORES = 8
C, H, W = 512, 128, 128
CQK = 64
KC = 4  # input-channel chunks of 128
OC = 4  # output-channel chunks of 128
NEG_INF = -1e9


def _dap(t, offset, dims):
    """Raw DRAM access pattern: dims = [(stride, count), ...] in elements."""
    a = t[...]
    return bass.AP(
        tensor=a.tensor, offset=a.offset + offset, ap=[[s, n] for s, n in dims]
    )


def build(n_cores: int = N_CORES, dbg: bool = False):
    nc = bacc.Bacc("TRN2", debug=False, num_devices=n_cores)

    x_d = nc.dram_tensor("x", [C, H, W], F32, kind="ExternalInput")
    wq_d = nc.dram_tensor("Wq", [CQK, C], F32, kind="ExternalInput")
    bq_d = nc.dram_tensor("bq", [CQK], F32, kind="ExternalInput")
    wk_d = nc.dram_tensor("Wk", [CQK, C], F32, kind="ExternalInput")
    bk_d = nc.dram_tensor("bk", [CQK], F32, kind="ExternalInput")
    wv_d = nc.dram_tensor("Wv", [C, C], F32, kind="ExternalInput")
    bv_d = nc.dram_tensor("bv", [C], F32, kind="ExternalInput")
    g_d = nc.dram_tensor("gamma", [1], F32, kind="ExternalInput")
    out_d = nc.dram_tensor("out", [C, H, W], F16, kind="ExternalOutput")

    v_scr = nc.dram_tensor(
        "v_scr", [H, W, C], F16, kind="ExternalOutput" if dbg else "Internal"
    )  # pixel-major
    nb_scr = nc.dram_tensor("nb_scr", [2, H * W], F16)
    if dbg:
        dbg_outs = {
            "dq": nc.dram_tensor("dq", [CQK + 2, H, W], F16, kind="ExternalOutput"),
            "dk": nc.dram_tensor("dk", [CQK + 2, H, W], F16, kind="ExternalOutput"),
            "ds1": nc.dram_tensor("ds1", [128, H], F32, kind="ExternalOutput"),
            "ds2": nc.dram_tensor("ds2", [128, W], F32, kind="ExternalOutput"),
            "dart": nc.dram_tensor("dart", [128, H, 128], F16, kind="ExternalOutput"),
            "dact": nc.dram_tensor("dact", [128, W, 128], F16, kind="ExternalOutput"),
            "dacc0": nc.dram_tensor("dacc0", [128, W, H], F16, kind="ExternalOutput"),
        }

    with tile.TileContext(nc) as tc, ExitStack() as ctx:
        const = ctx.enter_context(tc.tile_pool(name="const", bufs=1))
        stats = ctx.enter_context(tc.tile_pool(name="stats", bufs=1))

        # ---- constants ----------------------------------------------------
        ident32 = const.tile([128, 128], F32)
        from concourse.masks import make_identity

        make_identity(nc, ident32)
        ident16 = const.tile([128, 128], F16)
        nc.vector.tensor_copy(ident16, ident32)



        bq_sb = const.tile([CQK, 1], F32)
        nc.sync.dma_start(out=bq_sb, in_=bq_d[:].rearrange("(a b) -> a b", b=1))
        bk_sb = const.tile([CQK, 1], F32)
        nc.sync.dma_start(out=bk_sb, in_=bk_d[:].rearrange("(a b) -> a b", b=1))
        bv_sb = const.tile([128, OC], F32)
        nc.sync.dma_start(
            out=bv_sb, in_=bv_d[:].rearrange("(o p) -> p o", p=128)
        )
        g_ap = g_d[:]
        g_bcast = bass.AP(
            tensor=g_ap.tensor, offset=g_ap.offset, ap=[[0, 128], [1, 1]]
        )
        g_sb = const.tile([128, 1], F32)
        nc.gpsimd.dma_start(out=g_sb, in_=g_bcast)
        lng = stats.tile([128, 1], F32)
        nc.scalar.activation(lng, g_sb, AF.Ln)
        # gamma * bv, added in the final residual op
        gbv_sb = const.tile([128, OC], F32)
        nc.vector.tensor_scalar(
            out=gbv_sb, in0=bv_sb, scalar1=g_sb, scalar2=None, op0=ALU.mult
        )

        # transposed projection weights (fp16): wqkT [128, KC, 128] where
        # columns 0:64 = Wq^T chunk, 64:128 = Wk^T chunk; wvT [128, KC, 512]
        wqkT = const.tile([128, KC, 128], F16)
        wvT = const.tile([128, KC, C], F16)
        with tc.tile_pool(name="wprep", bufs=2) as wprep, tc.tile_pool(
            name="wps", bufs=2, space="PSUM"
        ) as wps:
            for kc in range(KC):
                for w_d, col0 in ((wq_d, 0), (wk_d, CQK)):
                    raw = wprep.tile([CQK, 128], F32, tag="rawqk")
                    nc.sync.dma_start(
                        out=raw, in_=w_d[:, kc * 128 : (kc + 1) * 128]
                    )
                    tps = wps.tile([128, CQK], F32, tag="tqk")
                    nc.tensor.transpose(tps, raw, ident32[:CQK, :CQK])
                    nc.vector.tensor_copy(
                        wqkT[:, kc, col0 : col0 + CQK], tps
                    )
                for oc in range(OC):
                    rawv = wprep.tile([128, 128], F32, tag="rawv")
                    nc.sync.dma_start(
                        out=rawv,
                        in_=wv_d[
                            oc * 128 : (oc + 1) * 128, kc * 128 : (kc + 1) * 128
                        ],
                    )
                    tps2 = wps.tile([128, 128], F32, tag="tv")
                    nc.tensor.transpose(tps2, rawv, ident32)
                    nc.vector.tensor_copy(
                        wvT[:, kc, oc * 128 : (oc + 1) * 128], tps2
                    )

        # ---- persistent attention maps (key-on-partition, fp16) ----------
        a_rowT = ctx.enter_context(tc.tile_pool(name="a_rowT", bufs=1))
        a_colT = ctx.enter_context(tc.tile_pool(name="a_colT", bufs=1))
        a_rowT_t = a_rowT.tile([128, H, 128], F16)  # (xk, y, xq)
        a_colT_t = a_colT.tile([128, W, 128], F16)  # (g,  x, yq)

        s1 = stats.tile([128, H], F32)  # [xq, y] row-branch exp sums
        s2 = stats.tile([128, W], F32)  # [yq, x] col-branch exp sums

        # ==================================================================
        # P1 + P2 in a nested scope so q/k free their SBUF before P3
        # ==================================================================
        with ExitStack() as p12:
            qk = p12.enter_context(tc.tile_pool(name="qk", bufs=1))
            # rows 0:64 = channels; rows 64,65 = nb hi/lo (q) and ones (k)
            q_sb = qk.tile([CQK + 2, H, W], F16)  # (c, y, x)
            k_sb = qk.tile([CQK + 2, H, W], F16)
            nc.gpsimd.memset(q_sb[CQK : CQK + 2, :, :], 0.0)
            nc.gpsimd.memset(k_sb[CQK : CQK + 2, :, :], 1.0)

            trash = p12.enter_context(tc.tile_pool(name="trash", bufs=4))

            # ---------------- P1: projections + row-branch Z sums ---------
            with tc.tile_pool(name="xin", bufs=3) as xin, tc.tile_pool(
                name="x16", bufs=3
            ) as x16p, tc.tile_pool(name="v16", bufs=2) as v16p, tc.tile_pool(
                name="p1ps", bufs=1, space="PSUM"
            ) as p1ps:
                for b in range(H // 4):
                    y0 = 4 * b
                    xt = xin.tile([128, KC, 512], F32, tag="xt")
                    for kc in range(KC):
                        nc.sync.dma_start(
                            out=xt[:, kc, :],
                            in_=x_d[
                                kc * 128 : (kc + 1) * 128, y0 : y0 + 4, :
                            ].rearrange("c r w -> c (r w)"),
                        )
                    x16 = x16p.tile([128, KC, 512], F16, tag="x16")
                    cast = nc.scalar.copy if b % 2 == 0 else nc.vector.tensor_copy
                    cast(
                        x16.rearrange("c k w -> c (k w)"),
                        xt.rearrange("c k w -> c (k w)"),
                    )


                    # q,k channel-major: psum [qk128, (4y,128x)]
                    qk_ps = p1ps.tile([128, 512], F32, tag="qkps", bufs=2)
                    for kc in range(KC):
                        nc.tensor.matmul(
                            qk_ps,
                            wqkT[:, kc, :],
                            x16[:, kc, :],
                            start=(kc == 0),
                            stop=(kc == KC - 1),
                        )
                    nc.vector.tensor_scalar_add(
                        q_sb[0:CQK, y0 : y0 + 4, :].rearrange(
                            "c r w -> c (r w)"
                        ),
                        qk_ps[0:CQK, :],
                        bq_sb,
                    )
                    nc.vector.tensor_scalar_add(
                        k_sb[0:CQK, y0 : y0 + 4, :].rearrange(
                            "c r w -> c (r w)"
                        ),
                        qk_ps[CQK:128, :],
                        bk_sb,
                    )

                    # v pixel-major: per row y, psum [x, c512]
                    v16 = v16p.tile([128, 4, C], F16, tag="v16")
                    for j in range(4):
                        v_ps = p1ps.tile([128, C], F32, tag="vps", bufs=4)
                        for kc in range(KC):
                            nc.tensor.matmul(
                                v_ps,
                                x16[:, kc, j * 128 : (j + 1) * 128],
                                wvT[:, kc, :],
                                start=(kc == 0),
                                stop=(kc == KC - 1),
                            )
                        vcp = (
                            nc.scalar.copy if j % 2 == 0 else nc.vector.tensor_copy
                        )
                        vcp(v16[:, j, :], v_ps)
                    nc.sync.dma_start(
                        out=_dap(
                            v_scr,
                            y0 * W * C,
                            [(C, 128), (W * C, 4), (1, C)],
                        ),
                        in_=v16,
                    )

                    # row-branch sum pass for this block (q aug rows are 0):
                    # rides P1's DMA stalls on otherwise-idle ACT/DVE slack
                    e_ps = p1ps.tile([128, 4, 128], F32, tag="eps", bufs=2)
                    for j in range(4):
                        nc.tensor.matmul(
                            e_ps[:, j, :],
                            q_sb[:, y0 + j, :],
                            k_sb[:, y0 + j, :],
                            start=True,
                            stop=True,
                        )
                    tr = trash.tile([128, 4, 128], F32, tag="trash")
                    nc.scalar.activation(
                        tr.rearrange("p a b -> p (a b)"),
                        e_ps.rearrange("p a b -> p (a b)"),
                        AF.Exp,
                    )
                    nc.vector.reduce_sum(s1[:, y0 : y0 + 4], tr, axis=AX.X)

            # ---------------- P2: softmax statistics ----------------------
            with tc.tile_pool(name="p2ps", bufs=1, space="PSUM") as p2ps:
                # ---- col-branch sum pass (query on partitions) -----------
                for x0 in range(0, W, 4):
                    e_ps = p2ps.tile([128, 4, 128], F32, tag="e_ps", bufs=4)
                    for j in range(4):
                        nc.tensor.matmul(
                            e_ps[:, j, :],
                            q_sb[:, :, x0 + j],
                            k_sb[:, :, x0 + j],
                            start=True,
                            stop=True,
                        )
                    tr = trash.tile([128, 4, 128], F32, tag="trash")
                    nc.scalar.activation(
                        tr.rearrange("p a b -> p (a b)"),
                        e_ps.rearrange("p a b -> p (a b)"),
                        AF.Exp,
                    )
                    # zero the self-pixel (diag yk == yq) before the reduce
                    nc.gpsimd.affine_select(
                        out=tr,
                        in_=tr,
                        compare_op=ALU.not_equal,
                        fill=0.0,
                        base=0,
                        pattern=[[0, 4], [-1, 128]],
                        channel_multiplier=1,
                    )
                    nc.vector.reduce_sum(s2[:, x0 : x0 + 4], tr, axis=AX.X)

                # ---- nb[y,x] = -(ln(Z) - ln(gamma)); ln via exponent
                # extraction so any fp32 Z is in the ACT Ln table range ----
                zt_ps = p2ps.tile([128, 128], F32, tag="zt", bufs=1)
                nc.tensor.transpose(zt_ps, s1, ident32)
                z_yx = stats.tile([128, W], F32)
                nc.vector.tensor_tensor(z_yx, zt_ps, s2, ALU.add)
                z_i = z_yx[...].bitcast(mybir.dt.int32)
                e_i32 = stats.tile([128, W], mybir.dt.int32)
                nc.vector.tensor_scalar(
                    out=e_i32,
                    in0=z_i,
                    scalar1=23,
                    scalar2=None,
                    op0=ALU.logical_shift_right,
                )
                ef = stats.tile([128, W], F32)
                nc.vector.tensor_scalar(
                    out=ef,
                    in0=e_i32,
                    scalar1=127,
                    scalar2=None,
                    op0=ALU.subtract,
                )
                mant = stats.tile([128, W], F32)
                nc.vector.tensor_scalar(
                    out=mant[...].bitcast(mybir.dt.int32),
                    in0=z_i,
                    scalar1=0x007FFFFF,
                    scalar2=0x3F800000,
                    op0=ALU.bitwise_and,
                    op1=ALU.bitwise_or,
                )
                lnm = stats.tile([128, W], F32)
                nc.scalar.activation(lnm, mant, AF.Ln)
                lnz = stats.tile([128, W], F32)
                nc.vector.scalar_tensor_tensor(
                    out=lnz,
                    in0=ef,
                    scalar=float(np.log(2.0)),
                    in1=lnm,
                    op0=ALU.mult,
                    op1=ALU.add,
                )
                nb_yx = stats.tile([128, W], F32)
                nc.vector.tensor_scalar(
                    out=nb_yx,
                    in0=lnz,
                    scalar1=lng,
                    scalar2=-1.0,
                    op0=ALU.subtract,
                    op1=ALU.mult,
                )
                # hi/lo fp16 split, bounced through DRAM into the two
                # augmented q partitions: e' = e + nb_hi + nb_lo
                nbh = stats.tile([128, W], F16)
                nc.vector.tensor_copy(nbh, nb_yx)
                nbh32 = stats.tile([128, W], F32)
                nc.vector.tensor_copy(nbh32, nbh)
                nbl = stats.tile([128, W], F16)
                nc.vector.tensor_tensor(nbl, nb_yx, nbh32, ALU.subtract)
                nc.sync.dma_start(
                    out=nb_scr[0:1, :].rearrange("o (y x) -> (o y) x", x=W),
                    in_=nbh,
                )
                nc.sync.dma_start(
                    out=nb_scr[1:2, :].rearrange("o (y x) -> (o y) x", x=W),
                    in_=nbl,
                )
                nc.sync.dma_start(
                    out=q_sb[CQK : CQK + 2, :, :].rearrange(
                        "c y x -> c (y x)"
                    ),
                    in_=nb_scr[:, :],
                )

                # ---- a passes: swapped operands (k stationary, q moving)
                # so psum comes out [key, query]; exp writes maps directly.
                # col pass first so P3's col branch can start earliest.
                for x0 in range(0, W, 4):
                    e_ps = p2ps.tile([128, 4, 128], F32, tag="e_ps", bufs=4)
                    for j in range(4):
                        nc.tensor.matmul(
                            e_ps[:, j, :],
                            k_sb[:, :, x0 + j],
                            q_sb[:, :, x0 + j],
                            start=True,
                            stop=True,
                        )
                    nc.scalar.activation(
                        a_colT_t[:, x0 : x0 + 4, :].rearrange(
                            "p a b -> p (a b)"
                        ),
                        e_ps.rearrange("p a b -> p (a b)"),
                        AF.Exp,
                    )
                    # zero the self-pixel (diag g == yq) in the stored map
                    nc.gpsimd.affine_select(
                        out=a_colT_t[:, x0 : x0 + 4, :],
                        in_=a_colT_t[:, x0 : x0 + 4, :],
                        compare_op=ALU.not_equal,
                        fill=0.0,
                        base=0,
                        pattern=[[0, 4], [-1, 128]],
                        channel_multiplier=1,
                    )
                for y0 in range(0, H, 4):
                    e_ps = p2ps.tile([128, 4, 128], F32, tag="e_ps", bufs=4)
                    for j in range(4):
                        nc.tensor.matmul(
                            e_ps[:, j, :],
                            k_sb[:, y0 + j, :],
                            q_sb[:, y0 + j, :],
                            start=True,
                            stop=True,
                        )
                    nc.scalar.activation(
                        a_rowT_t[:, y0 : y0 + 4, :].rearrange(
                            "p a b -> p (a b)"
                        ),
                        e_ps.rearrange("p a b -> p (a b)"),
                        AF.Exp,
                    )

            if dbg:
                for name, src in (
                    ("dq", q_sb),
                    ("dk", k_sb),
                    ("ds1", s1),
                    ("ds2", s2),
                    ("dart", a_rowT_t),
                    ("dact", a_colT_t),
                ):
                    d = dbg_outs[name]
                    nc.sync.dma_start(
                        out=d[...].rearrange("a b c -> a (b c)")
                        if len(d.shape) == 3
                        else d[...],
                        in_=src.rearrange("p a b -> p (a b)")
                        if len(src.shape) == 3
                        else src[0 : d.shape[0], :],
                    )

        # ==================================================================
        # P3: attention application, oc-pair at a time
        # ==================================================================
        with ExitStack() as p3:
            accp = p3.enter_context(tc.tile_pool(name="accp", bufs=1))
            vcolp = p3.enter_context(tc.tile_pool(name="vcolp", bufs=4))
            vrowp = p3.enter_context(tc.tile_pool(name="vrowp", bufs=4))
            xres = p3.enter_context(tc.tile_pool(name="xres", bufs=10))
            outp = p3.enter_context(tc.tile_pool(name="outp", bufs=4))

            with tc.tile_pool(name="p3ps", bufs=1, space="PSUM") as p3ps:
                for op in range(OC // 2):  # oc pair
                    oc0 = 2 * op
                    # --- col branch: acc[c', x, y] per oc in pair --------
                    accs = [
                        accp.tile(
                            [128, W, H], F16, tag=f"acc{s}", name=f"acc_{op}_{s}"
                        )
                        for s in range(2)
                    ]
                    for x0 in range(0, W, 8):
                        vc = vcolp.tile([128, 8, 256], F16, tag="vc", bufs=4)
                        nc.sync.dma_start(
                            out=vc,
                            in_=_dap(
                                v_scr,
                                x0 * C + oc0 * 128,
                                [(W * C, 128), (C, 8), (1, 256)],
                            ),
                        )
                        for xb in (x0, x0 + 4):
                            for s in range(2):
                                pc_ps = p3ps.tile(
                                    [128, 4, 128], F32, tag="pc", bufs=4
                                )
                                for j in range(4):
                                    nc.tensor.matmul(
                                        pc_ps[:, j, :],
                                        vc[
                                            :,
                                            xb - x0 + j,
                                            s * 128 : (s + 1) * 128,
                                        ],
                                        a_colT_t[:, xb + j, :],
                                        start=True,
                                        stop=True,
                                    )
                                ccp = (
                                    nc.scalar.copy
                                    if (xb // 4 + s) % 2 == 0
                                    else nc.vector.tensor_copy
                                )
                                ccp(
                                    accs[s][:, xb : xb + 4, :].rearrange(
                                        "c x y -> c (x y)"
                                    ),
                                    pc_ps.rearrange("c x y -> c (x y)"),
                                )

                    if dbg and op == 0:
                        nc.sync.dma_start(
                            out=dbg_outs["dacc0"][...].rearrange(
                                "a b c -> a (b c)"
                            ),
                            in_=accs[0].rearrange("p a b -> p (a b)"),
                        )
                    # --- row branch + combine + residual -----------------
                    # supergroups of 2 y4-blocks x 2 oc: the 4 fold matmuls
                    # run back-to-back (identity stationary loaded once)
                    for yg in range(0, H, 8):
                        vr8 = vrowp.tile([128, 8, 256], F16, tag="vr", bufs=4)
                        nc.sync.dma_start(
                            out=vr8,
                            in_=_dap(
                                v_scr,
                                yg * W * C + oc0 * 128,
                                [(C, 128), (W * C, 8), (1, 256)],
                            ),
                        )
                        vrs = [vr8[:, 0:4, :], vr8[:, 4:8, :]]
                        xrs = {}
                        for s in range(2):
                            oc = oc0 + s
                            for g in range(2):
                                y0 = yg + 4 * g
                                xr = xres.tile([128, 4, 128], F32, tag="xr")
                                nc.gpsimd.dma_start(
                                    out=xr.rearrange("c r w -> c (r w)"),
                                    in_=x_d[
                                        oc * 128 : (oc + 1) * 128,
                                        y0 : y0 + 4,
                                        :,
                                    ].rearrange("c r w -> c (r w)"),
                                )
                                xrs[(s, g)] = xr
                        prs = {}
                        for s in range(2):
                            for g in range(2):
                                y0 = yg + 4 * g
                                pr_ps = p3ps.tile(
                                    [128, 4, 128], F32, tag="pr", bufs=4,
                                    name=f"pr_{op}_{yg}_{s}_{g}",
                                )
                                acc_ap = accs[s][...]
                                acc_mov = bass.AP(
                                    tensor=acc_ap.tensor,
                                    offset=acc_ap.offset + y0,
                                    ap=[list(acc_ap.ap[0]), [1, 4], [H, W]],
                                )
                                nc.tensor.matmul(
                                    pr_ps.rearrange("c r w -> c (r w)"),
                                    ident16,
                                    acc_mov,
                                    start=True,
                                    stop=False,
                                    skip_group_check=True,
                                )
                                prs[(s, g)] = pr_ps
                        for s in range(2):
                            for g in range(2):
                                y0 = yg + 4 * g
                                pr_ps = prs[(s, g)]
                                for j in range(4):
                                    nc.tensor.matmul(
                                        pr_ps[:, j, :],
                                        vrs[g][:, j, s * 128 : (s + 1) * 128],
                                        a_rowT_t[:, y0 + j, :],
                                        start=False,
                                        stop=(j == 3),
                                        skip_group_check=True,
                                    )
                        for s in range(2):
                            oc = oc0 + s
                            for g in range(2):
                                y0 = yg + 4 * g
                                ot = outp.tile([128, 4, 128], F16, tag="ot")
                                nc.vector.scalar_tensor_tensor(
                                    out=ot.rearrange("c r w -> c (r w)"),
                                    in0=prs[(s, g)].rearrange(
                                        "c r w -> c (r w)"
                                    ),
                                    scalar=gbv_sb[:, oc : oc + 1],
                                    in1=xrs[(s, g)].rearrange(
                                        "c r w -> c (r w)"
                                    ),
                                    op0=ALU.add,
                                    op1=ALU.add,
                                )
                                nc.sync.dma_start(
                                    out=out_d[
                                        oc * 128 : (oc + 1) * 128,
                                        y0 : y0 + 4,
                                        :,
                                    ].rearrange("c r w -> c (r w)"),
                                    in_=ot.rearrange("p a b -> p (a b)"),
                                )

    nc.finalize()
    return nc


_NC_CACHE = {}


def _get_nc():
    if "nc" not in _NC_CACHE:
        _NC_CACHE["nc"] = build()
    return _NC_CACHE["nc"]


def kernel(**inputs) -> np.ndarray:
    x = np.ascontiguousarray(np.asarray(inputs["x"], dtype=np.float32))
    n = x.shape[0]
    assert x.shape == (n, C, H, W)
    shared = {
        name: np.ascontiguousarray(np.asarray(inputs[name], dtype=np.float32))
        for name in ("Wq", "bq", "Wk", "bk", "Wv", "bv", "gamma")
    }
    nc = _get_nc()
    in_maps = [{"x": x[i], **shared} for i in range(n)]
    res = run_bass_kernel_spmd(nc, in_maps, core_ids=list(range(n)))
    return np.stack(
        [res.results[i]["out"].astype(np.float32) for i in range(n)], axis=0
    )


if __name__ == "__main__":
    rng = np.random.default_rng(0)
    demo = {
        "x": rng.standard_normal((N_CORES, C, H, W), dtype=np.float32),
        "Wq": rng.standard_normal((CQK, C), dtype=np.float32) / np.sqrt(C),
        "bq": np.zeros(CQK, np.float32),
        "Wk": rng.standard_normal((CQK, C), dtype=np.float32) / np.sqrt(C),
        "bk": np.zeros(CQK, np.float32),
        "Wv": rng.standard_normal((C, C), dtype=np.float32) / np.sqrt(C),
        "bv": np.zeros(C, np.float32),
        "gamma": np.ones(1, np.float32),
    }
    out = kernel(**demo)
    print("out", out.shape, out.dtype, np.abs(out).mean())


# revision 41
# speedup vs baseline: 1.0493x; 1.0493x over previous
"""Criss-Cross Attention (CCA) Trainium2 Bass kernel — v2.

Problem: n=8 images of (c=512, h=128, w=128); per-pixel projections
q,k (64ch) and v (512ch); row + column attention with joint softmax over
the 256 (w + h) logits per pixel (self pixel masked out of the column
branch); out = gamma * att + x.

Sharding: data-parallel over batch — one image per NeuronCore (8 cores).

v2 design notes (all transposes eliminated):
  P1: stream x in 4-row blocks; project q,k channel-major into SBUF;
      project v PIXEL-major (x16 row-slice as matmul stationary, WvT as
      moving) giving [x, c512] tiles, stored to DRAM as v_scr[y][x][c].
  P2: sum pass (query-on-partition e-matmuls, exp, reduce) -> Z;
      nb = -(lnZ - ln gamma) folded into fp16 hi/lo aug rows of q_sb.
      a-pass with SWAPPED operands (k stationary, q moving) so the
      attention maps come out KEY-on-partition and are written straight
      to SBUF — no XBAR transpose DMAs.
  P3: per oc-pair: col branch (v column tiles read from v_scr as 512B
      lines, v-stationary matmuls) -> contiguous fp16 acc [c,x,y];
      row branch matmuls accumulate into PSUM, the col acc is folded in
      by an identity-matmul whose strided MOVING operand does the
      (x<->y) relabel for free on the PE; one fused DVE op adds the
      residual x and gamma*bv, then a straight DMA writes out.
"""

import sys

for _p in ("/opt/trn_rl_repo",):
    if _p not in sys.path:
        sys.path.insert(0, _p)

from contextlib import ExitStack

import numpy as np

from concourse import bacc
import concourse.bass as bass
import concourse.mybir as mybir
import concourse.tile as tile
from concourse.bass_utils import run_bass_kernel_spmd

F32 = mybir.dt.float32
F16 = mybir.dt.float16
AX = mybir.AxisListType
ALU = mybir.AluOpType
AF = mybir.ActivationFunctionType

N_CORES = 8
C, H, W = 512, 128, 128
CQK = 64
KC = 4  # input-channel chunks of 128
OC = 4  # output-channel chunks of 128
NEG_INF = -1e9


def _dap(t, offset, dims):
    """Raw DRAM access pattern: dims = [(stride, count), ...] in elements."""
    a = t[...]
    return bass.AP(
        tensor=a.tensor, offset=a.offset + offset, ap=[[s, n] for s, n in dims]
    )


def build(n_cores: int = N_CORES, dbg: bool = False):
    nc = bacc.Bacc("TRN2", debug=False, num_devices=n_cores)

    x_d = nc.dram_tensor("x", [C, H, W], F32, kind="ExternalInput")
    wq_d = nc.dram_tensor("Wq", [CQK, C], F32, kind="ExternalInput")
    bq_d = nc.dram_tensor("bq", [CQK], F32, kind="ExternalInput")
    wk_d = nc.dram_tensor("Wk", [CQK, C], F32, kind="ExternalInput")
    bk_d = nc.dram_tensor("bk", [CQK], F32, kind="ExternalInput")
    wv_d = nc.dram_tensor("Wv", [C, C], F32, kind="ExternalInput")
    bv_d = nc.dram_tensor("bv", [C], F32, kind="ExternalInput")
    g_d = nc.dram_tensor("gamma", [1], F32, kind="ExternalInput")
    out_d = nc.dram_tensor("out", [C, H, W], F16, kind="ExternalOutput")

    v_scr = nc.dram_tensor(
        "v_scr", [H, W, C], F16, kind="ExternalOutput" if dbg else "Internal"
    )  # pixel-major
    nb_scr = nc.dram_tensor("nb_scr", [2, H * W], F16)
    if dbg:
        dbg_outs = {
            "dq": nc.dram_tensor("dq", [CQK + 2, H, W], F16, kind="ExternalOutput"),
            "dk": nc.dram_tensor("dk", [CQK + 2, H, W], F16, kind="ExternalOutput"),
            "ds1": nc.dram_tensor("ds1", [128, H], F32, kind="ExternalOutput"),
            "ds2": nc.dram_tensor("ds2", [128, W], F32, kind="ExternalOutput"),
            "dart": nc.dram_tensor("dart", [128, H, 128], F16, kind="ExternalOutput"),
            "dact": nc.dram_tensor("dact", [128, W, 128], F16, kind="ExternalOutput"),
            "dacc0": nc.dram_tensor("dacc0", [128, W, H], F16, kind="ExternalOutput"),
        }

    with tile.TileContext(nc) as tc, ExitStack() as ctx:
        const = ctx.enter_context(tc.tile_pool(name="const", bufs=1))
        stats = ctx.enter_context(tc.tile_pool(name="stats", bufs=1))

        # ---- constants ----------------------------------------------------
        ident32 = const.tile([128, 128], F32)
        from concourse.masks import make_identity

        make_identity(nc, ident32)
        ident16 = const.tile([128, 128], F16)
        nc.vector.tensor_copy(ident16, ident32)



        bq_sb = const.tile([CQK, 1], F32)
        nc.sync.dma_start(out=bq_sb, in_=bq_d[:].rearrange("(a b) -> a b", b=1))
        bk_sb = const.tile([CQK, 1], F32)
        nc.sync.dma_start(out=bk_sb, in_=bk_d[:].rearrange("(a b) -> a b", b=1))
        bv_sb = const.tile([128, OC], F32)
        nc.sync.dma_start(
            out=bv_sb, in_=bv_d[:].rearrange("(o p) -> p o", p=128)
        )
        g_ap = g_d[:]
        g_bcast = bass.AP(
            tensor=g_ap.tensor, offset=g_ap.offset, ap=[[0, 128], [1, 1]]
        )
        g_sb = const.tile([128, 1], F32)
        nc.gpsimd.dma_start(out=g_sb, in_=g_bcast)
        lng = stats.tile([128, 1], F32)
        nc.scalar.activation(lng, g_sb, AF.Ln)
        # gamma * bv, added in the final residual op
        gbv_sb = const.tile([128, OC], F32)
        nc.vector.tensor_scalar(
            out=gbv_sb, in0=bv_sb, scalar1=g_sb, scalar2=None, op0=ALU.mult
        )

        # transposed projection weights (fp16): wqkT [128, KC, 128] where
        # columns 0:64 = Wq^T chunk, 64:128 = Wk^T chunk; wvT [128, KC, 512]
        wqkT = const.tile([128, KC, 128], F16)
        wvT = const.tile([128, KC, C], F16)
        with tc.tile_pool(name="wprep", bufs=2) as wprep, tc.tile_pool(
            name="wps", bufs=2, space="PSUM"
        ) as wps:
            for kc in range(KC):
                for w_d, col0 in ((wq_d, 0), (wk_d, CQK)):
                    raw = wprep.tile([CQK, 128], F32, tag="rawqk")
                    nc.sync.dma_start(
                        out=raw, in_=w_d[:, kc * 128 : (kc + 1) * 128]
                    )
                    tps = wps.tile([128, CQK], F32, tag="tqk")
                    nc.tensor.transpose(tps, raw, ident32[:CQK, :CQK])
                    nc.vector.tensor_copy(
                        wqkT[:, kc, col0 : col0 + CQK], tps
                    )
                for oc in range(OC):
                    rawv = wprep.tile([128, 128], F32, tag="rawv")
                    nc.sync.dma_start(
                        out=rawv,
                        in_=wv_d[
                            oc * 128 : (oc + 1) * 128, kc * 128 : (kc + 1) * 128
                        ],
                    )
                    tps2 = wps.tile([128, 128], F32, tag="tv")
                    nc.tensor.transpose(tps2, rawv, ident32)
                    nc.vector.tensor_copy(
                        wvT[:, kc, oc * 128 : (oc + 1) * 128], tps2
                    )

        # ---- persistent attention maps (key-on-partition, fp16) ----------
        a_rowT = ctx.enter_context(tc.tile_pool(name="a_rowT", bufs=1))
        a_colT = ctx.enter_context(tc.tile_pool(name="a_colT", bufs=1))
        a_rowT_t = a_rowT.tile([128, H, 128], F16)  # (xk, y, xq)
        a_colT_t = a_colT.tile([128, W, 128], F16)  # (g,  x, yq)

        s1 = stats.tile([128, H], F32)  # [xq, y] row-branch exp sums
        s2 = stats.tile([128, W], F32)  # [yq, x] col-branch exp sums

        # ==================================================================
        # P1 + P2 in a nested scope so q/k free their SBUF before P3
        # ==================================================================
        with ExitStack() as p12:
            qk = p12.enter_context(tc.tile_pool(name="qk", bufs=1))
            # rows 0:64 = channels; rows 64,65 = nb hi/lo (q) and ones (k)
            q_sb = qk.tile([CQK + 2, H, W], F16)  # (c, y, x)
            k_sb = qk.tile([CQK + 2, H, W], F16)
            nc.gpsimd.memset(q_sb[CQK : CQK + 2, :, :], 0.0)
            nc.gpsimd.memset(k_sb[CQK : CQK + 2, :, :], 1.0)

            trash = p12.enter_context(tc.tile_pool(name="trash", bufs=4))

            # ---------------- P1: projections + row-branch Z sums ---------
            with tc.tile_pool(name="xin", bufs=3) as xin, tc.tile_pool(
                name="x16", bufs=3
            ) as x16p, tc.tile_pool(name="v16", bufs=2) as v16p, tc.tile_pool(
                name="p1ps", bufs=1, space="PSUM"
            ) as p1ps:
                for b in range(H // 4):
                    y0 = 4 * b
                    xt = xin.tile([128, KC, 512], F32, tag="xt")
                    for kc in range(KC):
                        nc.sync.dma_start(
                            out=xt[:, kc, :],
                            in_=x_d[
                                kc * 128 : (kc + 1) * 128, y0 : y0 + 4, :
                            ].rearrange("c r w -> c (r w)"),
                        )
                    x16 = x16p.tile([128, KC, 512], F16, tag="x16")
                    cast = nc.scalar.copy if b % 2 == 0 else nc.vector.tensor_copy
                    cast(
                        x16.rearrange("c k w -> c (k w)"),
                        xt.rearrange("c k w -> c (k w)"),
                    )


                    # q,k channel-major: psum [qk128, (4y,128x)]
                    qk_ps = p1ps.tile([128, 512], F32, tag="qkps", bufs=2)
                    for kc in range(KC):
                        nc.tensor.matmul(
                            qk_ps,
                            wqkT[:, kc, :],
                            x16[:, kc, :],
                            start=(kc == 0),
                            stop=(kc == KC - 1),
                        )
                    nc.vector.tensor_scalar_add(
                        q_sb[0:CQK, y0 : y0 + 4, :].rearrange(
                            "c r w -> c (r w)"
                        ),
                        qk_ps[0:CQK, :],
                        bq_sb,
                    )
                    nc.vector.tensor_scalar_add(
                        k_sb[0:CQK, y0 : y0 + 4, :].rearrange(
                            "c r w -> c (r w)"
                        ),
                        qk_ps[CQK:128, :],
                        bk_sb,
                    )

                    # v pixel-major: per row y, psum [x, c512]
                    v16 = v16p.tile([128, 4, C], F16, tag="v16")
                    for j in range(4):
                        v_ps = p1ps.tile([128, C], F32, tag="vps", bufs=4)
                        for kc in range(KC):
                            nc.tensor.matmul(
                                v_ps,
                                x16[:, kc, j * 128 : (j + 1) * 128],
                                wvT[:, kc, :],
                                start=(kc == 0),
                                stop=(kc == KC - 1),
                            )
                        vcp = (
                            nc.scalar.copy if j % 2 == 0 else nc.vector.tensor_copy
                        )
                        vcp(v16[:, j, :], v_ps)
                    nc.sync.dma_start(
                        out=_dap(
                            v_scr,
                            y0 * W * C,
                            [(C, 128), (W * C, 4), (1, C)],
                        ),
                        in_=v16,
                    )

                    # row-branch sum pass for this block (q aug rows are 0):
                    # rides P1's DMA stalls on otherwise-idle ACT/DVE slack
                    e_ps = p1ps.tile([128, 4, 128], F32, tag="eps", bufs=2)
                    for j in range(4):
                        nc.tensor.matmul(
                            e_ps[:, j, :],
                            q_sb[:, y0 + j, :],
                            k_sb[:, y0 + j, :],
                            start=True,
                            stop=True,
                        )
                    tr = trash.tile([128, 4, 128], F32, tag="trash")
                    nc.scalar.activation(
                        tr.rearrange("p a b -> p (a b)"),
                        e_ps.rearrange("p a b -> p (a b)"),
                        AF.Exp,
                    )
                    nc.vector.reduce_sum(s1[:, y0 : y0 + 4], tr, axis=AX.X)

            # ---------------- P2: softmax statistics ----------------------
            with tc.tile_pool(name="p2ps", bufs=1, space="PSUM") as p2ps:
                # ---- col-branch sum pass (query on partitions) -----------
                for x0 in range(0, W, 4):
                    e_ps = p2ps.tile([128, 4, 128], F32, tag="e_ps", bufs=6)
                    for j in range(4):
                        nc.tensor.matmul(
                            e_ps[:, j, :],
                            q_sb[:, :, x0 + j],
                            k_sb[:, :, x0 + j],
                            start=True,
                            stop=True,
                        )
                    tr = trash.tile([128, 4, 128], F32, tag="trash")
                    nc.scalar.activation(
                        tr.rearrange("p a b -> p (a b)"),
                        e_ps.rearrange("p a b -> p (a b)"),
                        AF.Exp,
                    )
                    # zero the self-pixel (diag yk == yq) before the reduce
                    nc.gpsimd.affine_select(
                        out=tr,
                        in_=tr,
                        compare_op=ALU.not_equal,
                        fill=0.0,
                        base=0,
                        pattern=[[0, 4], [-1, 128]],
                        channel_multiplier=1,
                    )
                    nc.vector.reduce_sum(s2[:, x0 : x0 + 4], tr, axis=AX.X)

                # ---- nb[y,x] = -(ln(Z) - ln(gamma)); ln via exponent
                # extraction so any fp32 Z is in the ACT Ln table range ----
                zt_ps = p2ps.tile([128, 128], F32, tag="zt", bufs=1)
                nc.tensor.transpose(zt_ps, s1, ident32)
                z_yx = stats.tile([128, W], F32)
                nc.vector.tensor_tensor(z_yx, zt_ps, s2, ALU.add)
                z_i = z_yx[...].bitcast(mybir.dt.int32)
                e_i32 = stats.tile([128, W], mybir.dt.int32)
                nc.vector.tensor_scalar(
                    out=e_i32,
                    in0=z_i,
                    scalar1=23,
                    scalar2=None,
                    op0=ALU.logical_shift_right,
                )
                ef = stats.tile([128, W], F32)
                nc.vector.tensor_scalar(
                    out=ef,
                    in0=e_i32,
                    scalar1=127,
                    scalar2=None,
                    op0=ALU.subtract,
                )
                mant = stats.tile([128, W], F32)
                nc.vector.tensor_scalar(
                    out=mant[...].bitcast(mybir.dt.int32),
                    in0=z_i,
                    scalar1=0x007FFFFF,
                    scalar2=0x3F800000,
                    op0=ALU.bitwise_and,
                    op1=ALU.bitwise_or,
                )
                lnm = stats.tile([128, W], F32)
                nc.scalar.activation(lnm, mant, AF.Ln)
                lnz = stats.tile([128, W], F32)
                nc.vector.scalar_tensor_tensor(
                    out=lnz,
                    in0=ef,
                    scalar=float(np.log(2.0)),
                    in1=lnm,
                    op0=ALU.mult,
                    op1=ALU.add,
                )
                nb_yx = stats.tile([128, W], F32)
                nc.vector.tensor_scalar(
                    out=nb_yx,
                    in0=lnz,
                    scalar1=lng,
                    scalar2=-1.0,
                    op0=ALU.subtract,
                    op1=ALU.mult,
                )
                # hi/lo fp16 split, bounced through DRAM into the two
                # augmented q partitions: e' = e + nb_hi + nb_lo
                nbh = stats.tile([128, W], F16)
                nc.vector.tensor_copy(nbh, nb_yx)
                nbh32 = stats.tile([128, W], F32)
                nc.vector.tensor_copy(nbh32, nbh)
                nbl = stats.tile([128, W], F16)
                nc.vector.tensor_tensor(nbl, nb_yx, nbh32, ALU.subtract)
                nc.sync.dma_start(
                    out=nb_scr[0:1, :].rearrange("o (y x) -> (o y) x", x=W),
                    in_=nbh,
                )
                nc.sync.dma_start(
                    out=nb_scr[1:2, :].rearrange("o (y x) -> (o y) x", x=W),
                    in_=nbl,
                )
                nc.sync.dma_start(
                    out=q_sb[CQK : CQK + 2, :, :].rearrange(
                        "c y x -> c (y x)"
                    ),
                    in_=nb_scr[:, :],
                )

                # ---- a passes: swapped operands (k stationary, q moving)
                # so psum comes out [key, query]; exp writes maps directly.
                # col pass first so P3's col branch can start earliest.
                for x0 in range(0, W, 4):
                    e_ps = p2ps.tile([128, 4, 128], F32, tag="e_ps", bufs=6)
                    for j in range(4):
                        nc.tensor.matmul(
                            e_ps[:, j, :],
                            k_sb[:, :, x0 + j],
                            q_sb[:, :, x0 + j],
                            start=True,
                            stop=True,
                        )
                    nc.scalar.activation(
                        a_colT_t[:, x0 : x0 + 4, :].rearrange(
                            "p a b -> p (a b)"
                        ),
                        e_ps.rearrange("p a b -> p (a b)"),
                        AF.Exp,
                    )
                    # zero the self-pixel (diag g == yq) in the stored map
                    nc.gpsimd.affine_select(
                        out=a_colT_t[:, x0 : x0 + 4, :],
                        in_=a_colT_t[:, x0 : x0 + 4, :],
                        compare_op=ALU.not_equal,
                        fill=0.0,
                        base=0,
                        pattern=[[0, 4], [-1, 128]],
                        channel_multiplier=1,
                    )
                for y0 in range(0, H, 4):
                    e_ps = p2ps.tile([128, 4, 128], F32, tag="e_ps", bufs=6)
                    for j in range(4):
                        nc.tensor.matmul(
                            e_ps[:, j, :],
                            k_sb[:, y0 + j, :],
                            q_sb[:, y0 + j, :],
                            start=True,
                            stop=True,
                        )
                    nc.scalar.activation(
                        a_rowT_t[:, y0 : y0 + 4, :].rearrange(
                            "p a b -> p (a b)"
                        ),
                        e_ps.rearrange("p a b -> p (a b)"),
                        AF.Exp,
                    )

            if dbg:
                for name, src in (
                    ("dq", q_sb),
                    ("dk", k_sb),
                    ("ds1", s1),
                    ("ds2", s2),
                    ("dart", a_rowT_t),
                    ("dact", a_colT_t),
                ):
                    d = dbg_outs[name]
                    nc.sync.dma_start(
                        out=d[...].rearrange("a b c -> a (b c)")
                        if len(d.shape) == 3
                        else d[...],
                        in_=src.rearrange("p a b -> p (a b)")
                        if len(src.shape) == 3
                        else src[0 : d.shape[0], :],
                    )

        # ==================================================================
        # P3: attention application, oc-pair at a time
        # ==================================================================
        with ExitStack() as p3:
            accp = p3.enter_context(tc.tile_pool(name="accp", bufs=1))
            vcolp = p3.enter_context(tc.tile_pool(name="vcolp", bufs=4))
            vrowp = p3.enter_context(tc.tile_pool(name="vrowp", bufs=4))
            xres = p3.enter_context(tc.tile_pool(name="xres", bufs=10))
            outp = p3.enter_context(tc.tile_pool(name="outp", bufs=6))

            with tc.tile_pool(name="p3ps", bufs=1, space="PSUM") as p3ps:
                for op in range(OC // 2):  # oc pair
                    oc0 = 2 * op
                    # --- col branch: acc[c', x, y] per oc in pair --------
                    accs = [
                        accp.tile(
                            [128, W, H], F16, tag=f"acc{s}", name=f"acc_{op}_{s}"
                        )
                        for s in range(2)
                    ]
                    for x0 in range(0, W, 8):
                        vc = vcolp.tile([128, 8, 256], F16, tag="vc", bufs=4)
                        nc.sync.dma_start(
                            out=vc,
                            in_=_dap(
                                v_scr,
                                x0 * C + oc0 * 128,
                                [(W * C, 128), (C, 8), (1, 256)],
                            ),
                        )
                        for xb in (x0, x0 + 4):
                            for s in range(2):
                                pc_ps = p3ps.tile(
                                    [128, 4, 128], F32, tag="pc", bufs=4
                                )
                                for j in range(4):
                                    nc.tensor.matmul(
                                        pc_ps[:, j, :],
                                        vc[
                                            :,
                                            xb - x0 + j,
                                            s * 128 : (s + 1) * 128,
                                        ],
                                        a_colT_t[:, xb + j, :],
                                        start=True,
                                        stop=True,
                                    )
                                ccp = (
                                    nc.scalar.copy
                                    if (xb // 4 + s) % 2 == 0
                                    else nc.vector.tensor_copy
                                )
                                ccp(
                                    accs[s][:, xb : xb + 4, :].rearrange(
                                        "c x y -> c (x y)"
                                    ),
                                    pc_ps.rearrange("c x y -> c (x y)"),
                                )

                    if dbg and op == 0:
                        nc.sync.dma_start(
                            out=dbg_outs["dacc0"][...].rearrange(
                                "a b c -> a (b c)"
                            ),
                            in_=accs[0].rearrange("p a b -> p (a b)"),
                        )
                    # --- row branch + combine + residual -----------------
                    # supergroups of 2 y4-blocks x 2 oc: the 4 fold matmuls
                    # run back-to-back (identity stationary loaded once)
                    for yg in range(0, H, 8):
                        vr8 = vrowp.tile([128, 8, 256], F16, tag="vr", bufs=4)
                        nc.sync.dma_start(
                            out=vr8,
                            in_=_dap(
                                v_scr,
                                yg * W * C + oc0 * 128,
                                [(C, 128), (W * C, 8), (1, 256)],
                            ),
                        )
                        vrs = [vr8[:, 0:4, :], vr8[:, 4:8, :]]
                        xrs = {}
                        for s in range(2):
                            oc = oc0 + s
                            for g in range(2):
                                y0 = yg + 4 * g
                                xr = xres.tile([128, 4, 128], F32, tag="xr")
                                nc.gpsimd.dma_start(
                                    out=xr.rearrange("c r w -> c (r w)"),
                                    in_=x_d[
                                        oc * 128 : (oc + 1) * 128,
                                        y0 : y0 + 4,
                                        :,
                                    ].rearrange("c r w -> c (r w)"),
                                )
                                xrs[(s, g)] = xr
                        prs = {}
                        for s in range(2):
                            for g in range(2):
                                y0 = yg + 4 * g
                                pr_ps = p3ps.tile(
                                    [128, 4, 128], F32, tag="pr", bufs=4,
                                    name=f"pr_{op}_{yg}_{s}_{g}",
                                )
                                acc_ap = accs[s][...]
                                acc_mov = bass.AP(
                                    tensor=acc_ap.tensor,
                                    offset=acc_ap.offset + y0,
                                    ap=[list(acc_ap.ap[0]), [1, 4], [H, W]],
                                )
                                nc.tensor.matmul(
                                    pr_ps.rearrange("c r w -> c (r w)"),
                                    ident16,
                                    acc_mov,
                                    start=True,
                                    stop=False,
                                    skip_group_check=True,
                                )
                                prs[(s, g)] = pr_ps
                        for s in range(2):
                            for g in range(2):
                                y0 = yg + 4 * g
                                pr_ps = prs[(s, g)]
                                for j in range(4):
                                    nc.tensor.matmul(
                                        pr_ps[:, j, :],
                                        vrs[g][:, j, s * 128 : (s + 1) * 128],
                                        a_rowT_t[:, y0 + j, :],
                                        start=False,
                                        stop=(j == 3),
                                        skip_group_check=True,
                                    )
                        for s in range(2):
                            oc = oc0 + s
                            for g in range(2):
                                y0 = yg + 4 * g
                                ot = outp.tile([128, 4, 128], F16, tag="ot")
                                nc.vector.scalar_tensor_tensor(
                                    out=ot.rearrange("c r w -> c (r w)"),
                                    in0=prs[(s, g)].rearrange(
                                        "c r w -> c (r w)"
                                    ),
                                    scalar=gbv_sb[:, oc : oc + 1],
                                    in1=xrs[(s, g)].rearrange(
                                        "c r w -> c (r w)"
                                    ),
                                    op0=ALU.add,
                                    op1=ALU.add,
                                )
                                nc.sync.dma_start(
                                    out=out_d[
                                        oc * 128 : (oc + 1) * 128,
                                        y0 : y0 + 4,
                                        :,
                                    ].rearrange("c r w -> c (r w)"),
                                    in_=ot.rearrange("p a b -> p (a b)"),
                                )

    nc.finalize()
    return nc


_NC_CACHE = {}


def _get_nc():
    if "nc" not in _NC_CACHE:
        _NC_CACHE["nc"] = build()
    return _NC_CACHE["nc"]


def kernel(**inputs) -> np.ndarray:
    x = np.ascontiguousarray(np.asarray(inputs["x"], dtype=np.float32))
    n = x.shape[0]
    assert x.shape == (n, C, H, W)
    shared = {
        name: np.ascontiguousarray(np.asarray(inputs[name], dtype=np.float32))
        for name in ("Wq", "bq", "Wk", "bk", "Wv", "bv", "gamma")
    }
    nc = _get_nc()
    in_maps = [{"x": x[i], **shared} for i in range(n)]
    res = run_bass_kernel_spmd(nc, in_maps, core_ids=list(range(n)))
    return np.stack(
        [res.results[i]["out"].astype(np.float32) for i in range(n)], axis=0
    )


if __name__ == "__main__":
    rng = np.random.default_rng(0)
    demo = {
        "x": rng.standard_normal((N_CORES, C, H, W), dtype=np.float32),
        "Wq": rng.standard_normal((CQK, C), dtype=np.float32) / np.sqrt(C),
        "bq": np.zeros(CQK, np.float32),
        "Wk": rng.standard_normal((CQK, C), dtype=np.float32) / np.sqrt(C),
        "bk": np.zeros(CQK, np.float32),
        "Wv": rng.standard_normal((C, C), dtype=np.float32) / np.sqrt(C),
        "bv": np.zeros(C, np.float32),
        "gamma": np.ones(1, np.float32),
    }
    out = kernel(**demo)
    print("out", out.shape, out.dtype, np.abs(out).mean())


# revision 42
# speedup vs baseline: 1.0523x; 1.0029x over previous
"""Criss-Cross Attention (CCA) Trainium2 Bass kernel — v2.

Problem: n=8 images of (c=512, h=128, w=128); per-pixel projections
q,k (64ch) and v (512ch); row + column attention with joint softmax over
the 256 (w + h) logits per pixel (self pixel masked out of the column
branch); out = gamma * att + x.

Sharding: data-parallel over batch — one image per NeuronCore (8 cores).

v2 design notes (all transposes eliminated):
  P1: stream x in 4-row blocks; project q,k channel-major into SBUF;
      project v PIXEL-major (x16 row-slice as matmul stationary, WvT as
      moving) giving [x, c512] tiles, stored to DRAM as v_scr[y][x][c].
  P2: sum pass (query-on-partition e-matmuls, exp, reduce) -> Z;
      nb = -(lnZ - ln gamma) folded into fp16 hi/lo aug rows of q_sb.
      a-pass with SWAPPED operands (k stationary, q moving) so the
      attention maps come out KEY-on-partition and are written straight
      to SBUF — no XBAR transpose DMAs.
  P3: per oc-pair: col branch (v column tiles read from v_scr as 512B
      lines, v-stationary matmuls) -> contiguous fp16 acc [c,x,y];
      row branch matmuls accumulate into PSUM, the col acc is folded in
      by an identity-matmul whose strided MOVING operand does the
      (x<->y) relabel for free on the PE; one fused DVE op adds the
      residual x and gamma*bv, then a straight DMA writes out.
"""

import sys

for _p in ("/opt/trn_rl_repo",):
    if _p not in sys.path:
        sys.path.insert(0, _p)

from contextlib import ExitStack

import numpy as np

from concourse import bacc
import concourse.bass as bass
import concourse.mybir as mybir
import concourse.tile as tile
from concourse.bass_utils import run_bass_kernel_spmd

F32 = mybir.dt.float32
F16 = mybir.dt.float16
AX = mybir.AxisListType
ALU = mybir.AluOpType
AF = mybir.ActivationFunctionType

N_CORES = 8
C, H, W = 512, 128, 128
CQK = 64
KC = 4  # input-channel chunks of 128
OC = 4  # output-channel chunks of 128
NEG_INF = -1e9


def _dap(t, offset, dims):
    """Raw DRAM access pattern: dims = [(stride, count), ...] in elements."""
    a = t[...]
    return bass.AP(
        tensor=a.tensor, offset=a.offset + offset, ap=[[s, n] for s, n in dims]
    )


def build(n_cores: int = N_CORES, dbg: bool = False):
    nc = bacc.Bacc("TRN2", debug=False, num_devices=n_cores)

    x_d = nc.dram_tensor("x", [C, H, W], F32, kind="ExternalInput")
    wq_d = nc.dram_tensor("Wq", [CQK, C], F32, kind="ExternalInput")
    bq_d = nc.dram_tensor("bq", [CQK], F32, kind="ExternalInput")
    wk_d = nc.dram_tensor("Wk", [CQK, C], F32, kind="ExternalInput")
    bk_d = nc.dram_tensor("bk", [CQK], F32, kind="ExternalInput")
    wv_d = nc.dram_tensor("Wv", [C, C], F32, kind="ExternalInput")
    bv_d = nc.dram_tensor("bv", [C], F32, kind="ExternalInput")
    g_d = nc.dram_tensor("gamma", [1], F32, kind="ExternalInput")
    out_d = nc.dram_tensor("out", [C, H, W], F16, kind="ExternalOutput")

    v_scr = nc.dram_tensor(
        "v_scr", [H, W, C], F16, kind="ExternalOutput" if dbg else "Internal"
    )  # pixel-major
    nb_scr = nc.dram_tensor("nb_scr", [2, H * W], F16)
    if dbg:
        dbg_outs = {
            "dq": nc.dram_tensor("dq", [CQK + 2, H, W], F16, kind="ExternalOutput"),
            "dk": nc.dram_tensor("dk", [CQK + 2, H, W], F16, kind="ExternalOutput"),
            "ds1": nc.dram_tensor("ds1", [128, H], F32, kind="ExternalOutput"),
            "ds2": nc.dram_tensor("ds2", [128, W], F32, kind="ExternalOutput"),
            "dart": nc.dram_tensor("dart", [128, H, 128], F16, kind="ExternalOutput"),
            "dact": nc.dram_tensor("dact", [128, W, 128], F16, kind="ExternalOutput"),
            "dacc0": nc.dram_tensor("dacc0", [128, W, H], F16, kind="ExternalOutput"),
        }

    with tile.TileContext(nc) as tc, ExitStack() as ctx:
        const = ctx.enter_context(tc.tile_pool(name="const", bufs=1))
        stats = ctx.enter_context(tc.tile_pool(name="stats", bufs=1))

        # ---- constants ----------------------------------------------------
        ident32 = const.tile([128, 128], F32)
        from concourse.masks import make_identity

        make_identity(nc, ident32)
        ident16 = const.tile([128, 128], F16)
        nc.vector.tensor_copy(ident16, ident32)



        bq_sb = const.tile([CQK, 1], F32)
        nc.sync.dma_start(out=bq_sb, in_=bq_d[:].rearrange("(a b) -> a b", b=1))
        bk_sb = const.tile([CQK, 1], F32)
        nc.sync.dma_start(out=bk_sb, in_=bk_d[:].rearrange("(a b) -> a b", b=1))
        bv_sb = const.tile([128, OC], F32)
        nc.sync.dma_start(
            out=bv_sb, in_=bv_d[:].rearrange("(o p) -> p o", p=128)
        )
        g_ap = g_d[:]
        g_bcast = bass.AP(
            tensor=g_ap.tensor, offset=g_ap.offset, ap=[[0, 128], [1, 1]]
        )
        g_sb = const.tile([128, 1], F32)
        nc.gpsimd.dma_start(out=g_sb, in_=g_bcast)
        lng = stats.tile([128, 1], F32)
        nc.scalar.activation(lng, g_sb, AF.Ln)
        # gamma * bv, added in the final residual op
        gbv_sb = const.tile([128, OC], F32)
        nc.vector.tensor_scalar(
            out=gbv_sb, in0=bv_sb, scalar1=g_sb, scalar2=None, op0=ALU.mult
        )

        # transposed projection weights (fp16): wqkT [128, KC, 128] where
        # columns 0:64 = Wq^T chunk, 64:128 = Wk^T chunk; wvT [128, KC, 512]
        wqkT = const.tile([128, KC, 128], F16)
        wvT = const.tile([128, KC, C], F16)
        with tc.tile_pool(name="wprep", bufs=2) as wprep, tc.tile_pool(
            name="wps", bufs=2, space="PSUM"
        ) as wps:
            for kc in range(KC):
                for w_d, col0 in ((wq_d, 0), (wk_d, CQK)):
                    raw = wprep.tile([CQK, 128], F32, tag="rawqk")
                    nc.sync.dma_start(
                        out=raw, in_=w_d[:, kc * 128 : (kc + 1) * 128]
                    )
                    tps = wps.tile([128, CQK], F32, tag="tqk")
                    nc.tensor.transpose(tps, raw, ident32[:CQK, :CQK])
                    nc.vector.tensor_copy(
                        wqkT[:, kc, col0 : col0 + CQK], tps
                    )
                for oc in range(OC):
                    rawv = wprep.tile([128, 128], F32, tag="rawv")
                    nc.sync.dma_start(
                        out=rawv,
                        in_=wv_d[
                            oc * 128 : (oc + 1) * 128, kc * 128 : (kc + 1) * 128
                        ],
                    )
                    tps2 = wps.tile([128, 128], F32, tag="tv")
                    nc.tensor.transpose(tps2, rawv, ident32)
                    nc.vector.tensor_copy(
                        wvT[:, kc, oc * 128 : (oc + 1) * 128], tps2
                    )

        # ---- persistent attention maps (key-on-partition, fp16) ----------
        a_rowT = ctx.enter_context(tc.tile_pool(name="a_rowT", bufs=1))
        a_colT = ctx.enter_context(tc.tile_pool(name="a_colT", bufs=1))
        a_rowT_t = a_rowT.tile([128, H, 128], F16)  # (xk, y, xq)
        a_colT_t = a_colT.tile([128, W, 128], F16)  # (g,  x, yq)

        s1 = stats.tile([128, H], F32)  # [xq, y] row-branch exp sums
        s2 = stats.tile([128, W], F32)  # [yq, x] col-branch exp sums

        # ==================================================================
        # P1 + P2 in a nested scope so q/k free their SBUF before P3
        # ==================================================================
        with ExitStack() as p12:
            qk = p12.enter_context(tc.tile_pool(name="qk", bufs=1))
            # rows 0:64 = channels; rows 64,65 = nb hi/lo (q) and ones (k)
            q_sb = qk.tile([CQK + 2, H, W], F16)  # (c, y, x)
            k_sb = qk.tile([CQK + 2, H, W], F16)
            nc.gpsimd.memset(q_sb[CQK : CQK + 2, :, :], 0.0)
            nc.gpsimd.memset(k_sb[CQK : CQK + 2, :, :], 1.0)

            trash = p12.enter_context(tc.tile_pool(name="trash", bufs=6))

            # ---------------- P1: projections + row-branch Z sums ---------
            with tc.tile_pool(name="xin", bufs=3) as xin, tc.tile_pool(
                name="x16", bufs=4
            ) as x16p, tc.tile_pool(name="v16", bufs=2) as v16p, tc.tile_pool(
                name="p1ps", bufs=1, space="PSUM"
            ) as p1ps:
                for b in range(H // 4):
                    y0 = 4 * b
                    xt = xin.tile([128, KC, 512], F32, tag="xt")
                    for kc in range(KC):
                        nc.sync.dma_start(
                            out=xt[:, kc, :],
                            in_=x_d[
                                kc * 128 : (kc + 1) * 128, y0 : y0 + 4, :
                            ].rearrange("c r w -> c (r w)"),
                        )
                    x16 = x16p.tile([128, KC, 512], F16, tag="x16")
                    cast = nc.scalar.copy if b % 2 == 0 else nc.vector.tensor_copy
                    cast(
                        x16.rearrange("c k w -> c (k w)"),
                        xt.rearrange("c k w -> c (k w)"),
                    )


                    # q,k channel-major: psum [qk128, (4y,128x)]
                    qk_ps = p1ps.tile([128, 512], F32, tag="qkps", bufs=2)
                    for kc in range(KC):
                        nc.tensor.matmul(
                            qk_ps,
                            wqkT[:, kc, :],
                            x16[:, kc, :],
                            start=(kc == 0),
                            stop=(kc == KC - 1),
                        )
                    nc.vector.tensor_scalar_add(
                        q_sb[0:CQK, y0 : y0 + 4, :].rearrange(
                            "c r w -> c (r w)"
                        ),
                        qk_ps[0:CQK, :],
                        bq_sb,
                    )
                    nc.vector.tensor_scalar_add(
                        k_sb[0:CQK, y0 : y0 + 4, :].rearrange(
                            "c r w -> c (r w)"
                        ),
                        qk_ps[CQK:128, :],
                        bk_sb,
                    )

                    # v pixel-major: per row y, psum [x, c512]
                    v16 = v16p.tile([128, 4, C], F16, tag="v16")
                    for j in range(4):
                        v_ps = p1ps.tile([128, C], F32, tag="vps", bufs=4)
                        for kc in range(KC):
                            nc.tensor.matmul(
                                v_ps,
                                x16[:, kc, j * 128 : (j + 1) * 128],
                                wvT[:, kc, :],
                                start=(kc == 0),
                                stop=(kc == KC - 1),
                            )
                        vcp = (
                            nc.scalar.copy if j % 2 == 0 else nc.vector.tensor_copy
                        )
                        vcp(v16[:, j, :], v_ps)
                    nc.sync.dma_start(
                        out=_dap(
                            v_scr,
                            y0 * W * C,
                            [(C, 128), (W * C, 4), (1, C)],
                        ),
                        in_=v16,
                    )

                    # row-branch sum pass for this block (q aug rows are 0):
                    # rides P1's DMA stalls on otherwise-idle ACT/DVE slack
                    e_ps = p1ps.tile([128, 4, 128], F32, tag="eps", bufs=2)
                    for j in range(4):
                        nc.tensor.matmul(
                            e_ps[:, j, :],
                            q_sb[:, y0 + j, :],
                            k_sb[:, y0 + j, :],
                            start=True,
                            stop=True,
                        )
                    tr = trash.tile([128, 4, 128], F32, tag="trash")
                    nc.scalar.activation(
                        tr.rearrange("p a b -> p (a b)"),
                        e_ps.rearrange("p a b -> p (a b)"),
                        AF.Exp,
                    )
                    nc.vector.reduce_sum(s1[:, y0 : y0 + 4], tr, axis=AX.X)

            # ---------------- P2: softmax statistics ----------------------
            with tc.tile_pool(name="p2ps", bufs=1, space="PSUM") as p2ps:
                # ---- col-branch sum pass (query on partitions) -----------
                for x0 in range(0, W, 4):
                    e_ps = p2ps.tile([128, 4, 128], F32, tag="e_ps", bufs=6)
                    for j in range(4):
                        nc.tensor.matmul(
                            e_ps[:, j, :],
                            q_sb[:, :, x0 + j],
                            k_sb[:, :, x0 + j],
                            start=True,
                            stop=True,
                        )
                    tr = trash.tile([128, 4, 128], F32, tag="trash")
                    nc.scalar.activation(
                        tr.rearrange("p a b -> p (a b)"),
                        e_ps.rearrange("p a b -> p (a b)"),
                        AF.Exp,
                    )
                    # zero the self-pixel (diag yk == yq) before the reduce
                    nc.gpsimd.affine_select(
                        out=tr,
                        in_=tr,
                        compare_op=ALU.not_equal,
                        fill=0.0,
                        base=0,
                        pattern=[[0, 4], [-1, 128]],
                        channel_multiplier=1,
                    )
                    nc.vector.reduce_sum(s2[:, x0 : x0 + 4], tr, axis=AX.X)

                # ---- nb[y,x] = -(ln(Z) - ln(gamma)); ln via exponent
                # extraction so any fp32 Z is in the ACT Ln table range ----
                zt_ps = p2ps.tile([128, 128], F32, tag="zt", bufs=1)
                nc.tensor.transpose(zt_ps, s1, ident32)
                z_yx = stats.tile([128, W], F32)
                nc.vector.tensor_tensor(z_yx, zt_ps, s2, ALU.add)
                z_i = z_yx[...].bitcast(mybir.dt.int32)
                e_i32 = stats.tile([128, W], mybir.dt.int32)
                nc.vector.tensor_scalar(
                    out=e_i32,
                    in0=z_i,
                    scalar1=23,
                    scalar2=None,
                    op0=ALU.logical_shift_right,
                )
                ef = stats.tile([128, W], F32)
                nc.vector.tensor_scalar(
                    out=ef,
                    in0=e_i32,
                    scalar1=127,
                    scalar2=None,
                    op0=ALU.subtract,
                )
                mant = stats.tile([128, W], F32)
                nc.vector.tensor_scalar(
                    out=mant[...].bitcast(mybir.dt.int32),
                    in0=z_i,
                    scalar1=0x007FFFFF,
                    scalar2=0x3F800000,
                    op0=ALU.bitwise_and,
                    op1=ALU.bitwise_or,
                )
                lnm = stats.tile([128, W], F32)
                nc.scalar.activation(lnm, mant, AF.Ln)
                lnz = stats.tile([128, W], F32)
                nc.vector.scalar_tensor_tensor(
                    out=lnz,
                    in0=ef,
                    scalar=float(np.log(2.0)),
                    in1=lnm,
                    op0=ALU.mult,
                    op1=ALU.add,
                )
                nb_yx = stats.tile([128, W], F32)
                nc.vector.tensor_scalar(
                    out=nb_yx,
                    in0=lnz,
                    scalar1=lng,
                    scalar2=-1.0,
                    op0=ALU.subtract,
                    op1=ALU.mult,
                )
                # hi/lo fp16 split, bounced through DRAM into the two
                # augmented q partitions: e' = e + nb_hi + nb_lo
                nbh = stats.tile([128, W], F16)
                nc.vector.tensor_copy(nbh, nb_yx)
                nbh32 = stats.tile([128, W], F32)
                nc.vector.tensor_copy(nbh32, nbh)
                nbl = stats.tile([128, W], F16)
                nc.vector.tensor_tensor(nbl, nb_yx, nbh32, ALU.subtract)
                nc.sync.dma_start(
                    out=nb_scr[0:1, :].rearrange("o (y x) -> (o y) x", x=W),
                    in_=nbh,
                )
                nc.sync.dma_start(
                    out=nb_scr[1:2, :].rearrange("o (y x) -> (o y) x", x=W),
                    in_=nbl,
                )
                nc.sync.dma_start(
                    out=q_sb[CQK : CQK + 2, :, :].rearrange(
                        "c y x -> c (y x)"
                    ),
                    in_=nb_scr[:, :],
                )

                # ---- a passes: swapped operands (k stationary, q moving)
                # so psum comes out [key, query]; exp writes maps directly.
                # col pass first so P3's col branch can start earliest.
                for x0 in range(0, W, 4):
                    e_ps = p2ps.tile([128, 4, 128], F32, tag="e_ps", bufs=6)
                    for j in range(4):
                        nc.tensor.matmul(
                            e_ps[:, j, :],
                            k_sb[:, :, x0 + j],
                            q_sb[:, :, x0 + j],
                            start=True,
                            stop=True,
                        )
                    nc.scalar.activation(
                        a_colT_t[:, x0 : x0 + 4, :].rearrange(
                            "p a b -> p (a b)"
                        ),
                        e_ps.rearrange("p a b -> p (a b)"),
                        AF.Exp,
                    )
                    # zero the self-pixel (diag g == yq) in the stored map
                    nc.gpsimd.affine_select(
                        out=a_colT_t[:, x0 : x0 + 4, :],
                        in_=a_colT_t[:, x0 : x0 + 4, :],
                        compare_op=ALU.not_equal,
                        fill=0.0,
                        base=0,
                        pattern=[[0, 4], [-1, 128]],
                        channel_multiplier=1,
                    )
                for y0 in range(0, H, 4):
                    e_ps = p2ps.tile([128, 4, 128], F32, tag="e_ps", bufs=6)
                    for j in range(4):
                        nc.tensor.matmul(
                            e_ps[:, j, :],
                            k_sb[:, y0 + j, :],
                            q_sb[:, y0 + j, :],
                            start=True,
                            stop=True,
                        )
                    nc.scalar.activation(
                        a_rowT_t[:, y0 : y0 + 4, :].rearrange(
                            "p a b -> p (a b)"
                        ),
                        e_ps.rearrange("p a b -> p (a b)"),
                        AF.Exp,
                    )

            if dbg:
                for name, src in (
                    ("dq", q_sb),
                    ("dk", k_sb),
                    ("ds1", s1),
                    ("ds2", s2),
                    ("dart", a_rowT_t),
                    ("dact", a_colT_t),
                ):
                    d = dbg_outs[name]
                    nc.sync.dma_start(
                        out=d[...].rearrange("a b c -> a (b c)")
                        if len(d.shape) == 3
                        else d[...],
                        in_=src.rearrange("p a b -> p (a b)")
                        if len(src.shape) == 3
                        else src[0 : d.shape[0], :],
                    )

        # ==================================================================
        # P3: attention application, oc-pair at a time
        # ==================================================================
        with ExitStack() as p3:
            accp = p3.enter_context(tc.tile_pool(name="accp", bufs=1))
            vcolp = p3.enter_context(tc.tile_pool(name="vcolp", bufs=4))
            vrowp = p3.enter_context(tc.tile_pool(name="vrowp", bufs=4))
            xres = p3.enter_context(tc.tile_pool(name="xres", bufs=10))
            outp = p3.enter_context(tc.tile_pool(name="outp", bufs=6))

            with tc.tile_pool(name="p3ps", bufs=1, space="PSUM") as p3ps:
                for op in range(OC // 2):  # oc pair
                    oc0 = 2 * op
                    # --- col branch: acc[c', x, y] per oc in pair --------
                    accs = [
                        accp.tile(
                            [128, W, H], F16, tag=f"acc{s}", name=f"acc_{op}_{s}"
                        )
                        for s in range(2)
                    ]
                    for x0 in range(0, W, 8):
                        vc = vcolp.tile([128, 8, 256], F16, tag="vc", bufs=4)
                        nc.sync.dma_start(
                            out=vc,
                            in_=_dap(
                                v_scr,
                                x0 * C + oc0 * 128,
                                [(W * C, 128), (C, 8), (1, 256)],
                            ),
                        )
                        for xb in (x0, x0 + 4):
                            for s in range(2):
                                pc_ps = p3ps.tile(
                                    [128, 4, 128], F32, tag="pc", bufs=4
                                )
                                for j in range(4):
                                    nc.tensor.matmul(
                                        pc_ps[:, j, :],
                                        vc[
                                            :,
                                            xb - x0 + j,
                                            s * 128 : (s + 1) * 128,
                                        ],
                                        a_colT_t[:, xb + j, :],
                                        start=True,
                                        stop=True,
                                    )
                                ccp = (
                                    nc.scalar.copy
                                    if (xb // 4 + s) % 2 == 0
                                    else nc.vector.tensor_copy
                                )
                                ccp(
                                    accs[s][:, xb : xb + 4, :].rearrange(
                                        "c x y -> c (x y)"
                                    ),
                                    pc_ps.rearrange("c x y -> c (x y)"),
                                )

                    if dbg and op == 0:
                        nc.sync.dma_start(
                            out=dbg_outs["dacc0"][...].rearrange(
                                "a b c -> a (b c)"
                            ),
                            in_=accs[0].rearrange("p a b -> p (a b)"),
                        )
                    # --- row branch + combine + residual -----------------
                    # supergroups of 2 y4-blocks x 2 oc: the 4 fold matmuls
                    # run back-to-back (identity stationary loaded once)
                    for yg in range(0, H, 8):
                        vr8 = vrowp.tile([128, 8, 256], F16, tag="vr", bufs=4)
                        nc.sync.dma_start(
                            out=vr8,
                            in_=_dap(
                                v_scr,
                                yg * W * C + oc0 * 128,
                                [(C, 128), (W * C, 8), (1, 256)],
                            ),
                        )
                        vrs = [vr8[:, 0:4, :], vr8[:, 4:8, :]]
                        xrs = {}
                        for s in range(2):
                            oc = oc0 + s
                            for g in range(2):
                                y0 = yg + 4 * g
                                xr = xres.tile([128, 4, 128], F32, tag="xr")
                                nc.gpsimd.dma_start(
                                    out=xr.rearrange("c r w -> c (r w)"),
                                    in_=x_d[
                                        oc * 128 : (oc + 1) * 128,
                                        y0 : y0 + 4,
                                        :,
                                    ].rearrange("c r w -> c (r w)"),
                                )
                                xrs[(s, g)] = xr
                        prs = {}
                        for s in range(2):
                            for g in range(2):
                                y0 = yg + 4 * g
                                pr_ps = p3ps.tile(
                                    [128, 4, 128], F32, tag="pr", bufs=4,
                                    name=f"pr_{op}_{yg}_{s}_{g}",
                                )
                                acc_ap = accs[s][...]
                                acc_mov = bass.AP(
                                    tensor=acc_ap.tensor,
                                    offset=acc_ap.offset + y0,
                                    ap=[list(acc_ap.ap[0]), [1, 4], [H, W]],
                                )
                                nc.tensor.matmul(
                                    pr_ps.rearrange("c r w -> c (r w)"),
                                    ident16,
                                    acc_mov,
                                    start=True,
                                    stop=False,
                                    skip_group_check=True,
                                )
                                prs[(s, g)] = pr_ps
                        for s in range(2):
                            for g in range(2):
                                y0 = yg + 4 * g
                                pr_ps = prs[(s, g)]
                                for j in range(4):
                                    nc.tensor.matmul(
                                        pr_ps[:, j, :],
                                        vrs[g][:, j, s * 128 : (s + 1) * 128],
                                        a_rowT_t[:, y0 + j, :],
                                        start=False,
                                        stop=(j == 3),
                                        skip_group_check=True,
                                    )
                        for s in range(2):
                            oc = oc0 + s
                            for g in range(2):
                                y0 = yg + 4 * g
                                ot = outp.tile([128, 4, 128], F16, tag="ot")
                                nc.vector.scalar_tensor_tensor(
                                    out=ot.rearrange("c r w -> c (r w)"),
                                    in0=prs[(s, g)].rearrange(
                                        "c r w -> c (r w)"
                                    ),
                                    scalar=gbv_sb[:, oc : oc + 1],
                                    in1=xrs[(s, g)].rearrange(
                                        "c r w -> c (r w)"
                                    ),
                                    op0=ALU.add,
                                    op1=ALU.add,
                                )
                                nc.sync.dma_start(
                                    out=out_d[
                                        oc * 128 : (oc + 1) * 128,
                                        y0 : y0 + 4,
                                        :,
                                    ].rearrange("c r w -> c (r w)"),
                                    in_=ot.rearrange("p a b -> p (a b)"),
                                )

    nc.finalize()
    return nc


_NC_CACHE = {}


def _get_nc():
    if "nc" not in _NC_CACHE:
        _NC_CACHE["nc"] = build()
    return _NC_CACHE["nc"]


def kernel(**inputs) -> np.ndarray:
    x = np.ascontiguousarray(np.asarray(inputs["x"], dtype=np.float32))
    n = x.shape[0]
    assert x.shape == (n, C, H, W)
    shared = {
        name: np.ascontiguousarray(np.asarray(inputs[name], dtype=np.float32))
        for name in ("Wq", "bq", "Wk", "bk", "Wv", "bv", "gamma")
    }
    nc = _get_nc()
    in_maps = [{"x": x[i], **shared} for i in range(n)]
    res = run_bass_kernel_spmd(nc, in_maps, core_ids=list(range(n)))
    return np.stack(
        [res.results[i]["out"].astype(np.float32) for i in range(n)], axis=0
    )


if __name__ == "__main__":
    rng = np.random.default_rng(0)
    demo = {
        "x": rng.standard_normal((N_CORES, C, H, W), dtype=np.float32),
        "Wq": rng.standard_normal((CQK, C), dtype=np.float32) / np.sqrt(C),
        "bq": np.zeros(CQK, np.float32),
        "Wk": rng.standard_normal((CQK, C), dtype=np.float32) / np.sqrt(C),
        "bk": np.zeros(CQK, np.float32),
        "Wv": rng.standard_normal((C, C), dtype=np.float32) / np.sqrt(C),
        "bv": np.zeros(C, np.float32),
        "gamma": np.ones(1, np.float32),
    }
    out = kernel(**demo)
    print("out", out.shape, out.dtype, np.abs(out).mean())
